# revision 7
# baseline (speedup 1.0000x reference)
"""Attention kernel: baseline pipeline + startup DMA gating + boundary fixes.

On top of the gated startup: the exp activation-table load is warmed during
the DMA wait, and each phase's res_ps->res_sb copy is deferred into the next
phase's fill queue so the shared psum ring is not serialized across phase
boundaries.

All input DMAs used to be enqueued at once; the DMA engines round-robin
across queued transfers, so the critical loads (x16 + q/k/v weight gathers,
~2.5MB) co-finished with the non-critical 1.5MB (x f32 residual, w_out)
at ~17.6us, stalling the first matmul.  A pair of scribble ops (gated on
x16) creates WAW dependencies that hold the non-critical dma_starts back
until the critical transfers complete; the gated queues then stall at the
gated issue, serializing everything behind it.  The steady-state pipeline
(ScalarE-paced softmax with fused accums, fp16 matmuls, col-tiled AV
concurrency) is unchanged."""

import os
import sys

import numpy as np

for _p in ("/opt/trn_rl_repo",):
    if os.path.isdir(_p) and _p not in sys.path:
        sys.path.insert(0, _p)

import concourse.bass as bass
import concourse.mybir as mybir
import concourse.tile as tile
from concourse import bacc
from concourse.bass_utils import run_bass_kernel_spmd

F32 = mybir.dt.float32
FP16 = mybir.dt.float16
AF = mybir.ActivationFunctionType
ALU = mybir.AluOpType

N_HEADS = 8
DK = 64
C = 256
S = 1024
INNER = N_HEADS * DK  # 512
SCALE = DK ** -0.5
B = 8


def _body(nc, tc, ctx, x_d, wqkv_d, bqkv_d, wout_d, bout_d, y_d):
    sb = ctx.enter_context(tc.tile_pool(name="sb", bufs=1))
    sbP = ctx.enter_context(tc.tile_pool(name="sbP", bufs=1))
    ps = ctx.enter_context(tc.tile_pool(name="ps", bufs=1, space="PSUM"))

    x_sb = sb.tile([128, 2, S], F32)
    x16 = sb.tile([128, 2, S], FP16)
    wq16 = sb.tile([128, 2, INNER], FP16)
    wk16 = sb.tile([128, 2, INNER], FP16)
    wv16 = sb.tile([128, 2, INNER], FP16)
    wo16 = sb.tile([128, 4, C], FP16)
    qq_sb = sb.tile([128, 4, S], FP16)
    kk_sb = sb.tile([128, 4, S], FP16)
    v_sb = sb.tile([128, 8, INNER], FP16)
    res_sb = sb.tile([128, 4, S], FP16)
    out_sb = sb.tile([128, 2, S], F32)
    bq_sb = sb.tile([128, 4], F32)
    bk_sb = sb.tile([128, 4], F32)
    bv_row = sb.tile([1, INNER], FP16)
    ones_row = sb.tile([1, 128], FP16)
    bo_sb = sb.tile([128, 2], F32)
    s_sb = sb.tile([128, 64], F32)
    rs_sb = sb.tile([128, 64], F32)

    for ct in range(2):
        nc.gpsimd.dma_start(out=x16[:, ct, :],
                            in_=x_d[128 * ct:128 * (ct + 1), :])

    def w16_gather(off, wt):
        for ct in range(2):
            src = bass.AP(tensor=wqkv_d.tensor, offset=1536 * 128 * ct + off,
                          ap=[[1536, 128], [192, 8], [1, 64]])
            nc.gpsimd.dma_start(
                out=wt[:, ct, :].rearrange("p (h d) -> p h d", h=8, d=64),
                in_=src)
    w16_gather(0, wq16)
    w16_gather(64, wk16)
    w16_gather(128, wv16)
    bv_src = bass.AP(tensor=bqkv_d.tensor, offset=128, ap=[[192, 8], [1, 64]])
    nc.gpsimd.dma_start(
        out=bv_row[:, :].rearrange("p (h d) -> p h d", h=8, d=64), in_=bv_src)
    for off, btile in ((0, bq_sb), (64, bk_sb)):
        for hh in range(2):
            src = bass.AP(tensor=bqkv_d.tensor, offset=off + 192 * hh,
                          ap=[[1, 64], [384, 4]])
            nc.scalar.dma_start(out=btile[64 * hh:64 * (hh + 1), :], in_=src)
    bo_src = bass.AP(tensor=bout_d.tensor, offset=0, ap=[[1, 128], [128, 2]])
    nc.scalar.dma_start(out=bo_sb[:, :], in_=bo_src)
    # hold the non-critical loads back until x16 lands: the scribbles give
    # the first gated DMA on each queue a WAW dependency, and queue order
    # serializes the rest behind it
    nc.gpsimd.tensor_copy(out=wo16[0:1, 0, 0:8], in_=x16[0:1, 1, 0:8])
    nc.gpsimd.tensor_copy(out=x_sb[0:1, 0, 0:8], in_=x16[0:1, 1, 0:8])
    for ft in range(4):
        nc.gpsimd.dma_start(out=wo16[:, ft, :],
                            in_=wout_d[128 * ft:128 * (ft + 1), :])
    for ct in range(2):
        nc.sync.dma_start(out=x_sb[:, ct, :], in_=x_d[128 * ct:128 * (ct + 1), :])
    nc.vector.memset(ones_row[:, :], 1.0)
    # preload the exp activation table set (~1.3us) while input DMAs stream
    warm = sb.tile([1, 2], FP16)
    nc.scalar.activation(warm[:, :], ones_row[:, 0:2], AF.Exp)

    def emit_qk(p, t_idx, ih):
        wt, dst, btile = ((wq16, qq_sb, bq_sb), (wk16, kk_sb, bk_sb))[t_idx]
        g = ps.tile([128, 512], F32, tag="work", bufs=2,
                    name=f"qk_ps_{p}_{t_idx}_{ih}")
        for ct in range(2):
            nc.tensor.matmul(
                g[:, :],
                lhsT=wt[:, ct, 128 * p:128 * (p + 1)],
                rhs=x16[:, ct, 512 * ih:512 * (ih + 1)],
                start=(ct == 0), stop=(ct == 1),
            )
        nc.vector.tensor_scalar_add(
            out=dst[:, p, 512 * ih:512 * (ih + 1)], in0=g,
            scalar1=btile[:, p:p + 1],
        )

    def emit_v(tt):
        g = ps.tile([128, 512], F32, tag="work", bufs=2, name=f"v_ps_{tt}")
        for ct in range(2):
            nc.tensor.matmul(
                g[:, :],
                lhsT=x16[:, ct, 128 * tt:128 * (tt + 1)],
                rhs=wv16[:, ct, :],
                start=(ct == 0), stop=False,
            )
        nc.tensor.matmul(
            g[:, :], lhsT=ones_row[:, :], rhs=bv_row[:, :],
            start=False, stop=True,
        )
        nc.vector.tensor_copy(out=v_sb[:, tt, :], in_=g)

    def emit_out01(m, ih):
        g = ps.tile([128, 512], F32, tag="work", bufs=2, name=f"o01_{m}_{ih}")
        for ft in range(2):
            nc.tensor.matmul(
                g[:, :],
                lhsT=wo16[:, ft, 128 * m:128 * (m + 1)],
                rhs=res_sb[:, ft, 512 * ih:512 * (ih + 1)],
                start=(ft == 0), stop=(ft == 1),
            )
        nc.vector.tensor_tensor(
            out=out_sb[:, m, 512 * ih:512 * (ih + 1)], in0=g,
            in1=x_sb[:, m, 512 * ih:512 * (ih + 1)], op=ALU.add)

    for t_idx in range(2):
        emit_qk(0, t_idx, 0)
        emit_qk(0, t_idx, 1)

    fills = {
        0: [lambda tt=tt: emit_v(tt) for tt in range(8)]
           + [lambda ih=ih, t=t: emit_qk(1, t, ih)
              for ih in range(2) for t in range(2)],
        1: [lambda ih=ih, t=t: emit_qk(2, t, ih)
            for ih in range(2) for t in range(2)],
        2: [lambda ih=ih, t=t: emit_qk(3, t, ih)
            for ih in range(2) for t in range(2)]
           + [lambda m=m: emit_out01(m, 0) for m in range(2)],
        3: [lambda m=m: emit_out01(m, 1) for m in range(2)],
    }

    P_tiles = {}
    LAG = 3
    for p in range(4):
        res_ps = ps.tile([128, S], F32, tag="T", bufs=3, name=f"res_ps_{p}")
        fill = fills[p]
        for step in range(8 + LAG):
            J = step
            if J < 8:
                for hi in range(2):
                    h = 2 * p + hi
                    Tp = ps.tile([128, S], F32, tag="T", bufs=3, name=f"T_{h}_{J}")
                    for ih in range(2):
                        nc.tensor.matmul(
                            Tp[:, 512 * ih:512 * (ih + 1)],
                            lhsT=kk_sb[64 * hi:64 * hi + 64, p,
                                       128 * J:128 * (J + 1)],
                            rhs=qq_sb[64 * hi:64 * hi + 64, p,
                                      512 * ih:512 * (ih + 1)],
                            start=True, stop=True,
                        )
                    Pt = sbP.tile([128, S], FP16, tag="P", bufs=16,
                                  name=f"P_{h}_{J}")
                    c = 16 * p + 2 * J + hi
                    nc.scalar.activation(
                        Pt, Tp, AF.Exp, scale=SCALE,
                        accum_out=s_sb[:, c:c + 1],
                    )
                    P_tiles[(h, J)] = Pt
            Jn = step - 2
            if 0 <= Jn < 8:
                c0 = 16 * p + 2 * Jn
                nc.vector.reciprocal(rs_sb[:, c0:c0 + 2], s_sb[:, c0:c0 + 2])
                for hi in range(2):
                    h = 2 * p + hi
                    vs = v_sb[:, Jn, 64 * h:64 * h + 64]
                    nc.vector.tensor_scalar_mul(
                        out=vs, in0=vs, scalar1=rs_sb[:, c0 + hi:c0 + hi + 1])
            if fill:
                fill.pop(0)()
            Jav = step - LAG
            if Jav >= 0:
                for ih in range(2):
                    for hi in range(2):
                        h = 2 * p + hi
                        nc.tensor.matmul(
                            res_ps[64 * hi:64 * hi + 64, 512 * ih:512 * (ih + 1)],
                            lhsT=v_sb[:, Jav, 64 * h:64 * h + 64],
                            rhs=P_tiles[(h, Jav)][:, 512 * ih:512 * (ih + 1)],
                            start=(Jav == 0), stop=(Jav == 7),
                            skip_group_check=True,
                        )
        while fill:
            fill.pop(0)()
        # defer the res copy into the next phase's fill queue: it frees the
        # shared psum ring slot without stalling the boundary, since by the
        # time the next phase's first fill runs the last AV has drained
        def res_copy(p=p, res_ps=res_ps):
            nc.vector.tensor_copy(out=res_sb[:, p, :], in_=res_ps)
        if p < 3:
            fills[p + 1].insert(0, res_copy)
        else:
            res_copy()
        for J in range(8):
            for hi in range(2):
                del P_tiles[(2 * p + hi, J)]

    for m in range(2):
        for ih in range(2):
            g = ps.tile([128, 512], F32, tag="work", bufs=2, name=f"o23_{m}_{ih}")
            for ft in (2, 3):
                nc.tensor.matmul(
                    g[:, :],
                    lhsT=wo16[:, ft, 128 * m:128 * (m + 1)],
                    rhs=res_sb[:, ft, 512 * ih:512 * (ih + 1)],
                    start=(ft == 2), stop=(ft == 3),
                )
            nc.vector.scalar_tensor_tensor(
                out=out_sb[:, m, 512 * ih:512 * (ih + 1)],
                in0=g, scalar=bo_sb[:, m:m + 1],
                in1=out_sb[:, m, 512 * ih:512 * (ih + 1)],
                op0=ALU.add, op1=ALU.add,
            )
            nc.sync.dma_start(
                out=y_d[128 * m:128 * (m + 1), 512 * ih:512 * (ih + 1)],
                in_=out_sb[:, m, 512 * ih:512 * (ih + 1)])


_NC_CACHE = None


def _build_nc():
    global _NC_CACHE
    if _NC_CACHE is not None:
        return _NC_CACHE
    nc = bacc.Bacc("TRN2", target_bir_lowering=False)
    x_d = nc.dram_tensor("x", [C, S], F32, kind="ExternalInput")
    wqkv_d = nc.dram_tensor("w_qkv", [C, 3 * INNER], F32, kind="ExternalInput")
    bqkv_d = nc.dram_tensor("b_qkv", [3 * INNER], F32, kind="ExternalInput")
    wout_d = nc.dram_tensor("w_out", [INNER, C], F32, kind="ExternalInput")
    bout_d = nc.dram_tensor("b_out", [C], F32, kind="ExternalInput")
    y_d = nc.dram_tensor("y", [C, S], F32, kind="ExternalOutput")
    from contextlib import ExitStack
    with tile.TileContext(nc) as tc, ExitStack() as ctx:
        _body(nc, tc, ctx, x_d.ap(), wqkv_d.ap(), bqkv_d.ap(), wout_d.ap(),
              bout_d.ap(), y_d.ap())
    nc.compile()
    _NC_CACHE = nc
    return nc


def kernel(x, w_qkv, b_qkv, w_out, b_out, _trace=False, _tmpdir=None):
    x = np.ascontiguousarray(np.asarray(x, dtype=np.float32))
    w_qkv = np.ascontiguousarray(np.asarray(w_qkv, dtype=np.float32))
    b_qkv = np.ascontiguousarray(np.asarray(b_qkv, dtype=np.float32))
    w_out = np.ascontiguousarray(np.asarray(w_out, dtype=np.float32))
    b_out = np.ascontiguousarray(np.asarray(b_out, dtype=np.float32))

    nc = _build_nc()
    in_maps = [
        {
            "x": x[b].reshape(C, S),
            "w_qkv": w_qkv,
            "b_qkv": b_qkv,
            "w_out": w_out,
            "b_out": b_out,
        }
        for b in range(B)
    ]
    kw = {}
    if _trace:
        kw = {"trace": True, "tmpdir": _tmpdir}
    r = run_bass_kernel_spmd(nc, in_maps, core_ids=list(range(B)), **kw)
    y = np.stack([m["y"] for m in r.results], axis=0).reshape(B, C, 32, 32)
    if _trace:
        kernel.last_results = r
    return y
